# revision 8
# baseline (speedup 1.0000x reference)
"""Trainium2 Bass kernel for nn_Attention_72438918414857.

Reference computation (B=8, N=1024, C=768, H=12, D=64):
    qkv = (x @ qkv_w.T + qkv_b) -> q, k, v per head
    attn = softmax(q @ k.T / sqrt(D)) + static_a   (bias added AFTER softmax)
    out = (attn @ v) merged-heads @ proj_w.T + proj_b

Sharding: data-parallel over batch -- one batch element per NeuronCore,
weights + static_a replicated. No collectives needed.

Math used on-chip (per batch, per head), everything transposed so each
matmul gets its contraction dim on partitions with no on-chip transposes:
    qkT = [Wq;Wk]^T-proj of x  ->  [cout, t] layout
    E^T = exp(K_h^T.T @ Q_h^T * D^-0.5)           [k, q] strips
    out_h^T = ([V_h|1].T @ E^T) -> rows 0..63 = E@v, row 64 = rowsum(E)
    attn_h^T = (E@v) * (1/rowsum) + V_h.T @ A_h^T
where static_a is pre-transposed on host to A^T[h, k, q].  The softmax
normalization is applied to the [64, q] output instead of the [k, q]
matrix; no max-subtraction is needed (|scores*scale| < ~3).

Matmuls run in bf16 (fp32 PE matmul is 4x slower); PSUM accumulation is
fp32.  bf16 rounding of operands keeps rel-err ~1e-3, well under the
2e-2 gate.
"""

import os
import sys

import numpy as np

B, N, C = 8, 1024, 768
H, D = 12, 64
NCORES = 8
P = 128
QW = 512          # q tile width (PSUM bank = 512 f32)
NQT = N // QW     # 2 q tiles
NKT = N // P      # 8 k tiles
NCIN = C // P     # 6 c_in chunks
NPAIR = H // 2    # 6 head pairs
SCALE = float(D) ** -0.5

_REPO = "/opt/trn_rl_repo"


def _ensure_paths():
    if _REPO not in sys.path:
        sys.path.insert(0, _REPO)


def _split_excess_waits(nc):
    """The TRN2 walrus codegen allows only 1 sem-wait command per
    instruction.  Tile's sem-assigner can emit more (one per logical
    proc a tile depends on).
    Move the excess onto freshly inserted same-engine NoOps placed just
    before the instruction -- engines execute in order, so waiting on a
    preceding NoOp is equivalent."""
    import concourse.mybir as mybir
    from bass_rust import InstNoOp

    nid = [0]
    for fn in nc.m.functions:
        for blk in fn.blocks:
            out = []
            changed = False
            for inst in blk.instructions:
                si = inst.sync_info
                waits = list(si.on_wait) if si and si.on_wait else []
                limit = 1
                if len(waits) > limit:
                    extra, keep = waits[:-limit], waits[-limit:]
                    inst.sync_info = si.__replace__(on_wait=keep)
                    for w in extra:
                        nop = InstNoOp(
                            name=f"{inst.name}-wsplit{nid[0]}", ins=[], outs=[])
                        nid[0] += 1
                        nop.engine = inst.engine
                        nop.sync_info = mybir.SyncInfo(
                            on_wait=[w], on_update=[])
                        out.append(nop)
                    changed = True
                out.append(inst)
            if changed:
                blk.instructions = out


def build_nc():
    """Build the per-core Bass/Tile program."""
    _ensure_paths()
    import concourse.bass as bass
    import concourse.mybir as mybir
    import concourse.tile as tile
    from contextlib import ExitStack

    f32 = mybir.dt.float32
    bf16 = mybir.dt.bfloat16

    nc = bass.Bass("TRN2", target_bir_lowering=False, debug=False,
                   num_devices=NCORES)

    xT_ext = nc.declare_dram_parameter("xT", [C, N], f32, isOutput=False)
    qkwT_ext = nc.declare_dram_parameter("qkwT", [C, 2 * C], f32, isOutput=False)
    qkb_ext = nc.declare_dram_parameter("qkb", [P, 2 * C // P], f32, isOutput=False)
    vwT_ext = nc.declare_dram_parameter("vwT", [C, C], f32, isOutput=False)
    vb_ext = nc.declare_dram_parameter("vb", [1, C], f32, isOutput=False)
    at_ext = nc.declare_dram_parameter("at", [H, N, N], f32, isOutput=False)
    pwT_ext = nc.declare_dram_parameter("pwT", [C, C], f32, isOutput=False)
    pb_ext = nc.declare_dram_parameter("pb", [P, C // P], f32, isOutput=False)
    out_ext = nc.declare_dram_parameter("out", [C, N], f32, isOutput=True)

    NQK = 2 * C // P   # 12 cout tiles for q|k

    with tile.TileContext(nc, num_cores=NCORES) as tc, ExitStack() as ctx:
        consts = ctx.enter_context(tc.tile_pool(name="consts", bufs=1))
        persist = ctx.enter_context(tc.tile_pool(name="persist", bufs=1))
        attn_pool = ctx.enter_context(tc.tile_pool(name="attnout", bufs=1))

        qkb_sb = consts.tile([P, NQK], f32)
        nc.sync.dma_start(qkb_sb[:], qkb_ext[:])
        pb_sb = consts.tile([P, NCIN], f32)
        nc.sync.dma_start(pb_sb[:], pb_ext[:])
        vbf_sb = consts.tile([1, C], f32)
        nc.sync.dma_start(vbf_sb[:], vb_ext[:])
        vb_sb = consts.tile([1, C], bf16)
        nc.vector.tensor_copy(vb_sb[:], vbf_sb[:])
        ones_sb = consts.tile([1, P], bf16)
        nc.any.memset(ones_sb[:], 1.0)
        ones64_sb = consts.tile([1, 64], f32)
        nc.any.memset(ones64_sb[:], 1.0)

        # persistent activations (bf16 matmul operands)
        qkT_sb = persist.tile([P, NQK, N], bf16)      # [q|k]^T: cout x tokens
        vp_sb = persist.tile([P, H, NKT, 65], bf16)   # [V_h | 1] stationary
        nc.any.memset(vp_sb[:, :, :, 64:65], 1.0)
        attn_sb = attn_pool.tile([P, NCIN, N], bf16)  # attention out^T

        # ---------------- phase 1: qkv projections ----------------
        with tc.tile_pool(name="ph1", bufs=1) as ph1, \
             tc.tile_pool(name="pp_qk", bufs=2, space="PSUM") as pp_qk, \
             tc.tile_pool(name="pp_v", bufs=2, space="PSUM") as pp_v:
            xT_f = ph1.tile([P, NCIN, N], f32)
            nc.sync.dma_start(xT_f[:], xT_ext.rearrange("(c p) t -> p c t", p=P))
            xT_sb = ph1.tile([P, NCIN, N], bf16)
            nc.vector.tensor_copy(xT_sb[:], xT_f[:])

            qkw_f = ph1.tile([P, NCIN, 2 * C], f32)
            nc.sync.dma_start(
                qkw_f[:], qkwT_ext.rearrange("(c p) n -> p c n", p=P))
            qkw_sb = ph1.tile([P, NCIN, 2 * C], bf16)
            nc.vector.tensor_copy(qkw_sb[:], qkw_f[:])

            vw_f = ph1.tile([P, NCIN, C], f32)
            nc.sync.dma_start(vw_f[:], vwT_ext.rearrange("(c p) n -> p c n", p=P))
            vw_sb = ph1.tile([P, NCIN, C], bf16)
            nc.vector.tensor_copy(vw_sb[:], vw_f[:])

            # qkT[ct] = qkwT_slice.T @ xT  (+ per-partition bias, cast bf16)
            for ct in range(NQK):
                ps = pp_qk.tile([P, N], f32, tag="qk")
                for qh in range(NQT):
                    for kc in range(NCIN):
                        nc.tensor.matmul(
                            ps[:, qh * QW:(qh + 1) * QW],
                            qkw_sb[:, kc, ct * P:(ct + 1) * P],
                            xT_sb[:, kc, qh * QW:(qh + 1) * QW],
                            start=(kc == 0), stop=(kc == NCIN - 1))
                nc.vector.tensor_scalar_add(
                    qkT_sb[:, ct, :], ps[:, :], qkb_sb[:, ct:ct + 1])

            # V[tt] = xT_slice.T @ vwT (+ ones x vb rank-1 bias), cast bf16
            for tt in range(NKT):
                ps = pp_v.tile([P, C], f32, tag="v")
                for (n0, nw) in ((0, QW), (QW, C - QW)):
                    for kc in range(NCIN):
                        nc.tensor.matmul(
                            ps[:, n0:n0 + nw],
                            xT_sb[:, kc, tt * P:(tt + 1) * P],
                            vw_sb[:, kc, n0:n0 + nw],
                            start=(kc == 0), stop=False)
                    nc.tensor.matmul(
                        ps[:, n0:n0 + nw],
                        ones_sb[0:1, 0:P],
                        vb_sb[0:1, n0:n0 + nw],
                        start=False, stop=True)
                nc.vector.tensor_copy(
                    vp_sb[:, :, tt, 0:64],
                    ps.rearrange("p (h d) -> p h d", d=64))

        # ---------------- phase 2: attention ----------------
        with tc.tile_pool(name="epool", bufs=2) as epool, \
             tc.tile_pool(name="atpool", bufs=4) as atpool, \
             tc.tile_pool(name="atbf", bufs=4) as atbf, \
             tc.tile_pool(name="small", bufs=3) as small, \
             tc.tile_pool(name="pp_st", bufs=2, space="PSUM") as pp_st, \
             tc.tile_pool(name="pp_ev", bufs=2, space="PSUM") as pp_ev, \
             tc.tile_pool(name="pp_av", bufs=1, space="PSUM") as pp_av, \
             tc.tile_pool(name="pp_r", bufs=1, space="PSUM") as pp_r:
            for pr in range(NPAIR):
                h1, h2 = 2 * pr, 2 * pr + 1
                for qt in range(NQT):
                    q0 = qt * QW
                    # --- scores + exp: E^T strips [k, q] for both heads ---
                    e_sb = epool.tile([P, NKT, 2 * QW], bf16, tag="e")
                    for kt in range(NKT):
                        st = pp_st.tile([P, 2 * QW], f32, tag="st")
                        k0 = kt * P
                        nc.tensor.matmul(
                            st[:, 0:QW],
                            qkT_sb[0:64, NPAIR + pr, k0:k0 + P],
                            qkT_sb[0:64, pr, q0:q0 + QW],
                            start=True, stop=True)
                        nc.tensor.matmul(
                            st[:, QW:2 * QW],
                            qkT_sb[64:128, NPAIR + pr, k0:k0 + P],
                            qkT_sb[64:128, pr, q0:q0 + QW],
                            start=True, stop=True)
                        nc.scalar.activation(
                            e_sb[:, kt, :], st[:, :],
                            mybir.ActivationFunctionType.Exp, scale=SCALE)

                    # --- E@v (+rowsum via ones col) and A@v ---
                    psE1 = pp_ev.tile([65, QW], f32, tag="ev")
                    psE2 = pp_ev.tile([65, QW], f32, tag="ev")
                    psA = pp_av.tile([P, QW], f32, tag="av")
                    for kt in range(NKT):
                        at_f = atpool.tile([P, 2 * QW], f32, tag="at")
                        k0 = kt * P
                        nc.sync.dma_start(
                            at_f[:, 0:QW], at_ext[h1, k0:k0 + P, q0:q0 + QW])
                        nc.sync.dma_start(
                            at_f[:, QW:2 * QW], at_ext[h2, k0:k0 + P, q0:q0 + QW])
                        at = atbf.tile([P, 2 * QW], bf16, tag="atb")
                        nc.vector.tensor_copy(at[:], at_f[:])
                        st_flags = dict(start=(kt == 0), stop=(kt == NKT - 1))
                        nc.tensor.matmul(
                            psE1[:, :], vp_sb[:, h1, kt, :],
                            e_sb[:, kt, 0:QW], **st_flags)
                        nc.tensor.matmul(
                            psE2[:, :], vp_sb[:, h2, kt, :],
                            e_sb[:, kt, QW:2 * QW], **st_flags)
                        nc.tensor.matmul(
                            psA[0:64, :], vp_sb[:, h1, kt, 0:64],
                            at[:, 0:QW], **st_flags)
                        nc.tensor.matmul(
                            psA[64:128, :], vp_sb[:, h2, kt, 0:64],
                            at[:, QW:2 * QW], **st_flags)

                    # --- epilogue: out_h = E@v * (1/rowsum) + A@v ---
                    for hi, psE in ((0, psE1), (1, psE2)):
                        pa, pz = hi * 64, hi * 64 + 64
                        lns_sb = small.tile([1, QW], f32, tag="lns")
                        nc.scalar.activation(
                            lns_sb[:], psE[64:65, :],
                            mybir.ActivationFunctionType.Ln)
                        r_sb = small.tile([1, QW], f32, tag="r")
                        nc.scalar.activation(
                            r_sb[:], lns_sb[:],
                            mybir.ActivationFunctionType.Exp, scale=-1.0)
                        psR = pp_r.tile([64, QW], f32, tag="rp")
                        nc.tensor.matmul(psR[:, :], ones64_sb[:, :], r_sb[:, :],
                                         start=True, stop=True)
                        rb_sb = small.tile([64, QW], f32, tag="rb")
                        nc.vector.tensor_copy(rb_sb[:], psR[:, :])
                        dst = attn_sb[pa:pz, pr, q0:q0 + QW]
                        nc.vector.tensor_mul(dst, psE[0:64, :], rb_sb[:])
                        nc.vector.tensor_add(dst, dst, psA[pa:pz, :])

        # ---------------- phase 3: output projection ----------------
        with tc.tile_pool(name="ph3", bufs=1) as ph3, \
             tc.tile_pool(name="ph3o", bufs=2) as ph3o, \
             tc.tile_pool(name="pp_p", bufs=2, space="PSUM") as pp_p:
            pw_f = ph3.tile([P, NCIN, C], f32)
            nc.sync.dma_start(pw_f[:], pwT_ext.rearrange("(c p) n -> p c n", p=P))
            pw_sb = ph3.tile([P, NCIN, C], bf16)
            nc.vector.tensor_copy(pw_sb[:], pw_f[:])
            out_r = out_ext.rearrange("(c p) t -> p c t", p=P)
            for ct in range(NCIN):
                ps = pp_p.tile([P, N], f32, tag="pp")
                for qh in range(NQT):
                    for kc in range(NCIN):
                        nc.tensor.matmul(
                            ps[:, qh * QW:(qh + 1) * QW],
                            pw_sb[:, kc, ct * P:(ct + 1) * P],
                            attn_sb[:, kc, qh * QW:(qh + 1) * QW],
                            start=(kc == 0), stop=(kc == NCIN - 1))
                o_sb = ph3o.tile([P, N], f32, tag="o")
                nc.vector.tensor_scalar_add(o_sb[:], ps[:], pb_sb[:, ct:ct + 1])
                nc.sync.dma_start(out_r[:, ct, :], o_sb[:])

    _split_excess_waits(nc)
    return nc


def make_in_maps(x, qkv_w, qkv_b, static_a, proj_w, proj_b):
    """Host-side sharding / layout prep. One batch element per core."""
    x = np.asarray(x, dtype=np.float32)
    qkv_w = np.asarray(qkv_w, dtype=np.float32)
    qkv_b = np.asarray(qkv_b, dtype=np.float32)
    static_a = np.asarray(static_a, dtype=np.float32)
    proj_w = np.asarray(proj_w, dtype=np.float32)
    proj_b = np.asarray(proj_b, dtype=np.float32)

    qkwT = np.ascontiguousarray(qkv_w[0:2 * C].T)            # [768, 1536]
    qkb = np.ascontiguousarray(qkv_b[0:2 * C].reshape(2 * C // P, P).T)
    vwT = np.ascontiguousarray(qkv_w[2 * C:3 * C].T)         # [768, 768]
    vb = np.ascontiguousarray(qkv_b[2 * C:3 * C].reshape(1, C))
    at = np.ascontiguousarray(static_a[0].transpose(0, 2, 1))  # [H, k, q]
    pwT = np.ascontiguousarray(proj_w.T)
    pb = np.ascontiguousarray(proj_b.reshape(C // P, P).T)

    shared = {"qkwT": qkwT, "qkb": qkb, "vwT": vwT, "vb": vb,
              "at": at, "pwT": pwT, "pb": pb}
    in_maps = []
    for b in range(B):
        m = dict(shared)
        m["xT"] = np.ascontiguousarray(x[b].T)
        in_maps.append(m)
    return in_maps


_NC_CACHE = {}


def _get_nc():
    if "nc" not in _NC_CACHE:
        _NC_CACHE["nc"] = build_nc()
    return _NC_CACHE["nc"]


def kernel(x, qkv_w, qkv_b, static_a, proj_w, proj_b):
    _ensure_paths()
    from concourse.bass_utils import run_bass_kernel_spmd

    nc = _get_nc()
    in_maps = make_in_maps(x, qkv_w, qkv_b, static_a, proj_w, proj_b)
    res = run_bass_kernel_spmd(nc, in_maps, core_ids=list(range(NCORES)))
    out = np.empty((B, N, C), dtype=np.float32)
    for b in range(B):
        out[b] = res.results[b]["out"].T
    return out


# revision 9
# speedup vs baseline: 1.1548x; 1.1548x over previous
"""Trainium2 Bass kernel for nn_Attention_72438918414857.

Reference computation (B=8, N=1024, C=768, H=12, D=64):
    qkv = (x @ qkv_w.T + qkv_b) -> q, k, v per head
    attn = softmax(q @ k.T / sqrt(D)) + static_a   (bias added AFTER softmax)
    out = (attn @ v) merged-heads @ proj_w.T + proj_b

Sharding: data-parallel over batch -- one batch element per NeuronCore,
weights + static_a replicated. No collectives needed.

Math used on-chip (per batch, per head), everything transposed so each
matmul gets its contraction dim on partitions with no on-chip transposes:
    qkT = [Wq;Wk]^T-proj of x  ->  [cout, t] layout
    E^T = exp(K_h^T.T @ Q_h^T * D^-0.5)           [k, q] strips
    out_h^T = ([V_h|1].T @ E^T) -> rows 0..63 = E@v, row 64 = rowsum(E)
    attn_h^T = (E@v) * (1/rowsum) + V_h.T @ A_h^T
where static_a is pre-transposed on host to A^T[h, k, q].  The softmax
normalization is applied to the [64, q] output instead of the [k, q]
matrix; no max-subtraction is needed (|scores*scale| < ~3).

Matmuls run in bf16 (fp32 PE matmul is 4x slower); PSUM accumulation is
fp32.  bf16 rounding of operands keeps rel-err ~1e-3, well under the
2e-2 gate.
"""

import os
import sys

import numpy as np

B, N, C = 8, 1024, 768
H, D = 12, 64
NCORES = 8
P = 128
QW = 512          # q tile width (PSUM bank = 512 f32)
NQT = N // QW     # 2 q tiles
NKT = N // P      # 8 k tiles
NCIN = C // P     # 6 c_in chunks
NPAIR = H // 2    # 6 head pairs
SCALE = float(D) ** -0.5

_REPO = "/opt/trn_rl_repo"


def _ensure_paths():
    if _REPO not in sys.path:
        sys.path.insert(0, _REPO)


def _split_excess_waits(nc):
    """The TRN2 walrus codegen allows only 1 sem-wait command per
    instruction.  Tile's sem-assigner can emit more (one per logical
    proc a tile depends on).
    Move the excess onto freshly inserted same-engine NoOps placed just
    before the instruction -- engines execute in order, so waiting on a
    preceding NoOp is equivalent."""
    import concourse.mybir as mybir
    from bass_rust import InstNoOp

    nid = [0]
    for fn in nc.m.functions:
        for blk in fn.blocks:
            out = []
            changed = False
            for inst in blk.instructions:
                si = inst.sync_info
                waits = list(si.on_wait) if si and si.on_wait else []
                limit = 1
                if len(waits) > limit:
                    extra, keep = waits[:-limit], waits[-limit:]
                    inst.sync_info = si.__replace__(on_wait=keep)
                    for w in extra:
                        nop = InstNoOp(
                            name=f"{inst.name}-wsplit{nid[0]}", ins=[], outs=[])
                        nid[0] += 1
                        nop.engine = inst.engine
                        nop.sync_info = mybir.SyncInfo(
                            on_wait=[w], on_update=[])
                        out.append(nop)
                    changed = True
                out.append(inst)
            if changed:
                blk.instructions = out


def build_nc():
    """Build the per-core Bass/Tile program."""
    _ensure_paths()
    import concourse.bass as bass
    import concourse.mybir as mybir
    import concourse.tile as tile
    from contextlib import ExitStack

    f32 = mybir.dt.float32
    bf16 = mybir.dt.bfloat16

    nc = bass.Bass("TRN2", target_bir_lowering=False, debug=False,
                   num_devices=NCORES)

    xT_ext = nc.declare_dram_parameter("xT", [C, N], f32, isOutput=False)
    qkwT_ext = nc.declare_dram_parameter("qkwT", [C, 2 * C], f32, isOutput=False)
    qkb_ext = nc.declare_dram_parameter("qkb", [P, 2 * C // P], f32, isOutput=False)
    vwT_ext = nc.declare_dram_parameter("vwT", [C, C], f32, isOutput=False)
    vb_ext = nc.declare_dram_parameter("vb", [1, C], f32, isOutput=False)
    at_ext = nc.declare_dram_parameter(
        "at", [NPAIR, NQT, NKT, P, 2 * QW], f32, isOutput=False)
    pwT_ext = nc.declare_dram_parameter("pwT", [C, C], f32, isOutput=False)
    pb_ext = nc.declare_dram_parameter("pb", [P, C // P], f32, isOutput=False)
    out_ext = nc.declare_dram_parameter("out", [C, N], f32, isOutput=True)

    NQK = 2 * C // P   # 12 cout tiles for q|k

    with tile.TileContext(nc, num_cores=NCORES) as tc, ExitStack() as ctx:
        consts = ctx.enter_context(tc.tile_pool(name="consts", bufs=1))
        persist = ctx.enter_context(tc.tile_pool(name="persist", bufs=1))
        attn_pool = ctx.enter_context(tc.tile_pool(name="attnout", bufs=1))

        qkb_sb = consts.tile([P, NQK], f32)
        nc.sync.dma_start(qkb_sb[:], qkb_ext[:])
        pb_sb = consts.tile([P, NCIN], f32)
        nc.sync.dma_start(pb_sb[:], pb_ext[:])
        vbf_sb = consts.tile([1, C], f32)
        nc.sync.dma_start(vbf_sb[:], vb_ext[:])
        vb_sb = consts.tile([1, C], bf16)
        nc.vector.tensor_copy(vb_sb[:], vbf_sb[:])
        ones_sb = consts.tile([1, P], bf16)
        nc.any.memset(ones_sb[:], 1.0)
        ones64_sb = consts.tile([1, 64], f32)
        nc.any.memset(ones64_sb[:], 1.0)

        # persistent activations (bf16 matmul operands)
        qkT_sb = persist.tile([P, NQK, N], bf16)      # [q|k]^T: cout x tokens
        vp_sb = persist.tile([P, H, NKT, 65], bf16)   # [V_h | 1] stationary
        nc.any.memset(vp_sb[:, :, :, 64:65], 1.0)
        pw_sb = persist.tile([P, NCIN, C], bf16)      # proj weights (bf16)
        attn_sb = attn_pool.tile([P, NCIN, N], bf16)  # attention out^T

        # ---------------- phase 1: qkv projections ----------------
        with tc.tile_pool(name="ph1", bufs=2) as ph1, \
             tc.tile_pool(name="pp_qk", bufs=2, space="PSUM") as pp_qk, \
             tc.tile_pool(name="pp_v", bufs=2, space="PSUM") as pp_v:
            xT_sb = ph1.tile([P, NCIN, N], bf16)
            qkw_sb = ph1.tile([P, NCIN, 2 * C], bf16)
            vw_sb = ph1.tile([P, NCIN, C], bf16)
            # staged f32 loads (double-buffered) casted into bf16 tensors,
            # so matmuls can start before all weights have landed
            loads = [
                (xT_ext.rearrange("(c p) t -> p c t", p=P), xT_sb[:], N),
                (qkwT_ext.rearrange("(c p) n -> p c n", p=P)[:, :, 0:C],
                 qkw_sb[:, :, 0:C], C),
                (qkwT_ext.rearrange("(c p) n -> p c n", p=P)[:, :, C:2 * C],
                 qkw_sb[:, :, C:2 * C], C),
                (vwT_ext.rearrange("(c p) n -> p c n", p=P), vw_sb[:], C),
                (pwT_ext.rearrange("(c p) n -> p c n", p=P), pw_sb[:], C),
            ]
            for src_ap, dst_ap, w in loads:
                stg = ph1.tile([P, NCIN, N], f32, tag="stage")
                nc.sync.dma_start(stg[:, :, 0:w], src_ap)
                nc.vector.tensor_copy(dst_ap, stg[:, :, 0:w])

            # qkT[ct] = qkwT_slice.T @ xT  (+ per-partition bias, cast bf16)
            for ct in range(NQK):
                ps = pp_qk.tile([P, N], f32, tag="qk")
                for qh in range(NQT):
                    for kc in range(NCIN):
                        nc.tensor.matmul(
                            ps[:, qh * QW:(qh + 1) * QW],
                            qkw_sb[:, kc, ct * P:(ct + 1) * P],
                            xT_sb[:, kc, qh * QW:(qh + 1) * QW],
                            start=(kc == 0), stop=(kc == NCIN - 1))
                nc.vector.tensor_scalar_add(
                    qkT_sb[:, ct, :], ps[:, :], qkb_sb[:, ct:ct + 1])

            # V[tt] = xT_slice.T @ vwT (+ ones x vb rank-1 bias), cast bf16
            for tt in range(NKT):
                ps = pp_v.tile([P, C], f32, tag="v")
                for (n0, nw) in ((0, QW), (QW, C - QW)):
                    for kc in range(NCIN):
                        nc.tensor.matmul(
                            ps[:, n0:n0 + nw],
                            xT_sb[:, kc, tt * P:(tt + 1) * P],
                            vw_sb[:, kc, n0:n0 + nw],
                            start=(kc == 0), stop=False)
                    nc.tensor.matmul(
                        ps[:, n0:n0 + nw],
                        ones_sb[0:1, 0:P],
                        vb_sb[0:1, n0:n0 + nw],
                        start=False, stop=True)
                nc.vector.tensor_copy(
                    vp_sb[:, :, tt, 0:64],
                    ps.rearrange("p (h d) -> p h d", d=64))

        # ---------------- phase 2: attention ----------------
        with tc.tile_pool(name="epool", bufs=2) as epool, \
             tc.tile_pool(name="atpool", bufs=16) as atpool, \
             tc.tile_pool(name="atbf", bufs=12) as atbf, \
             tc.tile_pool(name="small", bufs=3) as small, \
             tc.tile_pool(name="pp_st", bufs=2, space="PSUM") as pp_st, \
             tc.tile_pool(name="pp_ev", bufs=2, space="PSUM") as pp_ev, \
             tc.tile_pool(name="pp_av", bufs=1, space="PSUM") as pp_av, \
             tc.tile_pool(name="pp_r", bufs=1, space="PSUM") as pp_r:
            for pr in range(NPAIR):
                h1, h2 = 2 * pr, 2 * pr + 1
                for qt in range(NQT):
                    q0 = qt * QW
                    # --- scores + exp: E^T strips [k, q] for both heads ---
                    e_sb = epool.tile([P, NKT, 2 * QW], bf16, tag="e")
                    for kt in range(NKT):
                        st = pp_st.tile([P, 2 * QW], f32, tag="st")
                        k0 = kt * P
                        nc.tensor.matmul(
                            st[:, 0:QW],
                            qkT_sb[0:64, NPAIR + pr, k0:k0 + P],
                            qkT_sb[0:64, pr, q0:q0 + QW],
                            start=True, stop=True)
                        nc.tensor.matmul(
                            st[:, QW:2 * QW],
                            qkT_sb[64:128, NPAIR + pr, k0:k0 + P],
                            qkT_sb[64:128, pr, q0:q0 + QW],
                            start=True, stop=True)
                        nc.scalar.activation(
                            e_sb[:, kt, :], st[:, :],
                            mybir.ActivationFunctionType.Exp, scale=SCALE)

                    # --- E@v (+rowsum via ones col) and A@v ---
                    psE1 = pp_ev.tile([65, QW], f32, tag="ev")
                    psE2 = pp_ev.tile([65, QW], f32, tag="ev")
                    psA = pp_av.tile([P, QW], f32, tag="av")
                    for kt in range(NKT):
                        at_f = atpool.tile([P, 2 * QW], f32, tag="at")
                        nc.sync.dma_start(at_f[:], at_ext[pr, qt, kt])
                        at = atbf.tile([P, 2 * QW], bf16, tag="atb")
                        nc.vector.tensor_copy(at[:], at_f[:])
                        st_flags = dict(start=(kt == 0), stop=(kt == NKT - 1))
                        nc.tensor.matmul(
                            psE1[:, :], vp_sb[:, h1, kt, :],
                            e_sb[:, kt, 0:QW], **st_flags)
                        nc.tensor.matmul(
                            psE2[:, :], vp_sb[:, h2, kt, :],
                            e_sb[:, kt, QW:2 * QW], **st_flags)
                        nc.tensor.matmul(
                            psA[0:64, :], vp_sb[:, h1, kt, 0:64],
                            at[:, 0:QW], **st_flags)
                        nc.tensor.matmul(
                            psA[64:128, :], vp_sb[:, h2, kt, 0:64],
                            at[:, QW:2 * QW], **st_flags)

                    # --- epilogue: out_h = E@v * (1/rowsum) + A@v ---
                    for hi, psE in ((0, psE1), (1, psE2)):
                        pa, pz = hi * 64, hi * 64 + 64
                        lns_sb = small.tile([1, QW], f32, tag="lns")
                        nc.scalar.activation(
                            lns_sb[:], psE[64:65, :],
                            mybir.ActivationFunctionType.Ln)
                        r_sb = small.tile([1, QW], f32, tag="r")
                        nc.scalar.activation(
                            r_sb[:], lns_sb[:],
                            mybir.ActivationFunctionType.Exp, scale=-1.0)
                        psR = pp_r.tile([64, QW], f32, tag="rp")
                        nc.tensor.matmul(psR[:, :], ones64_sb[:, :], r_sb[:, :],
                                         start=True, stop=True)
                        rb_sb = small.tile([64, QW], f32, tag="rb")
                        nc.vector.tensor_copy(rb_sb[:], psR[:, :])
                        dst = attn_sb[pa:pz, pr, q0:q0 + QW]
                        nc.vector.tensor_mul(dst, psE[0:64, :], rb_sb[:])
                        nc.vector.tensor_add(dst, dst, psA[pa:pz, :])

        # ---------------- phase 3: output projection ----------------
        with tc.tile_pool(name="ph3o", bufs=2) as ph3o, \
             tc.tile_pool(name="pp_p", bufs=2, space="PSUM") as pp_p:
            out_r = out_ext.rearrange("(c p) t -> p c t", p=P)
            for ct in range(NCIN):
                ps = pp_p.tile([P, N], f32, tag="pp")
                for qh in range(NQT):
                    for kc in range(NCIN):
                        nc.tensor.matmul(
                            ps[:, qh * QW:(qh + 1) * QW],
                            pw_sb[:, kc, ct * P:(ct + 1) * P],
                            attn_sb[:, kc, qh * QW:(qh + 1) * QW],
                            start=(kc == 0), stop=(kc == NCIN - 1))
                o_sb = ph3o.tile([P, N], f32, tag="o")
                nc.vector.tensor_scalar_add(o_sb[:], ps[:], pb_sb[:, ct:ct + 1])
                nc.sync.dma_start(out_r[:, ct, :], o_sb[:])

    _split_excess_waits(nc)
    return nc


def make_in_maps(x, qkv_w, qkv_b, static_a, proj_w, proj_b):
    """Host-side sharding / layout prep. One batch element per core."""
    x = np.asarray(x, dtype=np.float32)
    qkv_w = np.asarray(qkv_w, dtype=np.float32)
    qkv_b = np.asarray(qkv_b, dtype=np.float32)
    static_a = np.asarray(static_a, dtype=np.float32)
    proj_w = np.asarray(proj_w, dtype=np.float32)
    proj_b = np.asarray(proj_b, dtype=np.float32)

    qkwT = np.ascontiguousarray(qkv_w[0:2 * C].T)            # [768, 1536]
    qkb = np.ascontiguousarray(qkv_b[0:2 * C].reshape(2 * C // P, P).T)
    vwT = np.ascontiguousarray(qkv_w[2 * C:3 * C].T)         # [768, 768]
    vb = np.ascontiguousarray(qkv_b[2 * C:3 * C].reshape(1, C))
    # A^T strips, contiguous per (pair, qtile, ktile): [6, 2, 8, 128, 1024]
    # at[pr, qt, kt, :, 0:512] = A^T[2pr][kt tile, qt tile], [..., 512:] = head 2pr+1
    atT = static_a[0].transpose(0, 2, 1)                      # [H, k, q]
    at = np.ascontiguousarray(
        atT.reshape(NPAIR, 2, NKT, P, NQT, QW).transpose(0, 4, 2, 3, 1, 5)
        .reshape(NPAIR, NQT, NKT, P, 2 * QW))
    pwT = np.ascontiguousarray(proj_w.T)
    pb = np.ascontiguousarray(proj_b.reshape(C // P, P).T)

    shared = {"qkwT": qkwT, "qkb": qkb, "vwT": vwT, "vb": vb,
              "at": at, "pwT": pwT, "pb": pb}
    in_maps = []
    for b in range(B):
        m = dict(shared)
        m["xT"] = np.ascontiguousarray(x[b].T)
        in_maps.append(m)
    return in_maps


_NC_CACHE = {}


def _get_nc():
    if "nc" not in _NC_CACHE:
        _NC_CACHE["nc"] = build_nc()
    return _NC_CACHE["nc"]


def kernel(x, qkv_w, qkv_b, static_a, proj_w, proj_b):
    _ensure_paths()
    from concourse.bass_utils import run_bass_kernel_spmd

    nc = _get_nc()
    in_maps = make_in_maps(x, qkv_w, qkv_b, static_a, proj_w, proj_b)
    res = run_bass_kernel_spmd(nc, in_maps, core_ids=list(range(NCORES)))
    out = np.empty((B, N, C), dtype=np.float32)
    for b in range(B):
        out[b] = res.results[b]["out"].T
    return out
